# revision 2
# baseline (speedup 1.0000x reference)
"""Adaptive embedding lookup (4 vocab buckets, per-bucket projection) on 8 TRN2 cores.

Strategy: token-parallel SPMD. Tokens are bucketed by vocab range on host, each
bucket's vocab is split into <=32768-row chunks (int16 gather indices), each
group's token list is padded to a multiple of 8 and dealt round-robin so all 8
cores get identical static shapes. On device, each core:
  - dma_gather(transpose=True) pulls 128 tokens' bf16 embedding rows per tile,
    landing pre-transposed (embed dim on partitions, tokens on free dim) —
    directly usable as matmul lhsT, no on-chip transposes
  - matmuls against host-pre-transposed, EMB_SCALE-folded bf16 projections
  - writes its [rows, 1024] f32 output shard contiguously (group-major)
Host then inverse-permutes the 8 shards into the full [B, S, 1024] output.

Tables are converted to bf16 with embed dim zero-padded up to a multiple of
128 (dma_gather transpose needs 256B-aligned rows; zero dims contribute
nothing to the matmul).
"""
import sys

import numpy as np

if "/opt/trn_rl_repo" not in sys.path:
    sys.path.insert(0, "/opt/trn_rl_repo")

import ml_dtypes  # noqa: E402
from concourse import bacc, mybir, tile  # noqa: E402
from concourse.bass_utils import run_bass_kernel_spmd  # noqa: E402

N_CORES = 8
P = 128
CUTS = [0, 20000, 40000, 200000, 267735]
N_BUCKETS = 4
D_PROJ = 1024
EMB_SCALE = float(D_PROJ) ** 0.5
CHUNK = 32768  # max rows per gather table (int16 index range)

F32 = mybir.dt.float32
BF16 = mybir.dt.bfloat16
I16 = mybir.dt.int16


def _cdiv(a, b):
    return -(-a // b)


def _build_graph(groups, T):
    """groups: list of dicts with keys bucket, rows, d_pad, kc, m, n_tiles."""
    R = sum(g["m"] for g in groups)
    nc = bacc.Bacc(None, target_bir_lowering=False, debug=False)

    idx_p = nc.declare_dram_parameter("idx", [P, 8 * T], I16, isOutput=False)
    for gi, g in enumerate(groups):
        g["emb_p"] = nc.declare_dram_parameter(
            f"emb{gi}", [g["rows"], g["d_pad"]], BF16, isOutput=False
        )
    # one projT param per bucket, laid out [128, kc*D_PROJ]
    pt_p = {}
    kc_b = {}
    for g in groups:
        kc_b[g["bucket"]] = g["kc"]
    for b, kc in kc_b.items():
        pt_p[b] = nc.declare_dram_parameter(
            f"pt{b}", [P, kc * D_PROJ], BF16, isOutput=False
        )
    out_p = nc.declare_dram_parameter("out", [R, D_PROJ], F32, isOutput=True)

    with tile.TileContext(nc) as tc:
        with (
            tc.tile_pool(name="persist", bufs=1) as pp,
            tc.tile_pool(name="gather", bufs=3) as gp,
            tc.tile_pool(name="outs", bufs=3) as op,
            tc.tile_pool(name="ps_mm", bufs=6, space="PSUM") as ps_mm,
        ):
            idx_sb = pp.tile([P, 8 * T], I16)
            nc.sync.dma_start(out=idx_sb[:], in_=idx_p[:])

            pt_sb = {}
            for b, kc in kc_b.items():
                t_ = pp.tile([P, kc * D_PROJ], BF16, tag=f"pt{b}")
                nc.scalar.dma_start(out=t_[:], in_=pt_p[b][:])
                pt_sb[b] = t_

            t = 0
            row_start = 0
            for g in groups:
                b, d_pad, kc = g["bucket"], g["d_pad"], g["kc"]
                for j in range(g["n_tiles"]):
                    rows = min(P, g["m"] - j * P)
                    gt = gp.tile([P, kc, P], BF16, tag=f"g{kc}")
                    nc.gpsimd.dma_gather(
                        out_ap=gt[:],
                        in_ap=g["emb_p"][:],
                        idxs_ap=idx_sb[:, t * 8 : (t + 1) * 8],
                        num_idxs=P,
                        num_idxs_reg=P,
                        elem_size=d_pad,
                        transpose=True,
                    )
                    out_sb = op.tile([P, D_PROJ], F32, tag="o")
                    for h in range(D_PROJ // 512):
                        mm = ps_mm.tile([P, 512], F32, tag="mm")
                        for k in range(kc):
                            nc.tensor.matmul(
                                mm[:, :],
                                gt[:, k, :],
                                pt_sb[b][:, k * D_PROJ + h * 512 : k * D_PROJ + (h + 1) * 512],
                                start=(k == 0),
                                stop=(k == kc - 1),
                            )
                        if h == 0:
                            nc.vector.tensor_copy(
                                out=out_sb[:, h * 512 : (h + 1) * 512], in_=mm[:, :]
                            )
                        else:
                            nc.scalar.activation(
                                out=out_sb[:, h * 512 : (h + 1) * 512],
                                in_=mm[:, :],
                                func=mybir.ActivationFunctionType.Copy,
                            )
                    nc.sync.dma_start(
                        out=out_p[row_start : row_start + rows, :],
                        in_=out_sb[:rows, :],
                    )
                    row_start += rows
                    t += 1

    nc.compile()
    return nc


def kernel(inp, emb0, emb1, emb2, emb3, proj0, proj1, proj2, proj3):
    embs = [np.asarray(e, dtype=np.float32) for e in (emb0, emb1, emb2, emb3)]
    projs = [proj0, proj1, proj2, proj3]
    d_emb = [e.shape[1] for e in embs]
    v_emb = [e.shape[0] for e in embs]

    inp = np.asarray(inp)
    orig_shape = inp.shape
    flat = inp.reshape(-1).astype(np.int64)
    N = flat.shape[0]

    bucket = np.digitize(flat, CUTS[1:-1])  # 0..3
    local = flat - np.asarray(CUTS, dtype=np.int64)[bucket]

    # bf16 tables, embed dim zero-padded to a multiple of 128
    embs_bf = []
    d_pads = []
    for b in range(N_BUCKETS):
        d = d_emb[b]
        d_pad = max(P, _cdiv(d, P) * P)
        e = embs[b].astype(ml_dtypes.bfloat16)
        if d_pad != d:
            e = np.concatenate(
                [e, np.zeros((v_emb[b], d_pad - d), dtype=ml_dtypes.bfloat16)], axis=1
            )
        embs_bf.append(np.ascontiguousarray(e))
        d_pads.append(d_pad)

    # groups: (bucket, vocab chunk) — process small-projection buckets first
    # so the big proj0 load overlaps earlier compute
    groups = []
    for b in (1, 2, 3, 0):
        for base in range(0, v_emb[b], CHUNK):
            rows = min(CHUNK, v_emb[b] - base)
            sel = np.nonzero((bucket == b) & (local >= base) & (local < base + rows))[0]
            n = len(sel)
            npad = max(N_CORES, _cdiv(n, N_CORES) * N_CORES)
            pos_full = np.full(npad, -1, dtype=np.int64)
            loc_full = np.zeros(npad, dtype=np.int64)
            pos_full[:n] = sel
            loc_full[:n] = local[sel] - base
            m = npad // N_CORES
            groups.append(
                dict(
                    bucket=b,
                    base=base,
                    rows=rows,
                    d_pad=d_pads[b],
                    kc=d_pads[b] // P,
                    m=m,
                    n_tiles=_cdiv(m, P),
                    pos=pos_full,
                    loc=loc_full,
                )
            )
    T = sum(g["n_tiles"] for g in groups)

    # per-core int16 index arrays in dma_gather's wrapped layout:
    # gather slot i of tile t -> partition i%16, column t*8 + i//16,
    # 16-row block replicated 8x down the 128 partitions
    idx_arrs = []
    for c in range(N_CORES):
        cols = np.zeros((16, 8 * T), dtype=np.int16)
        t = 0
        for g in groups:
            loc_c = g["loc"][c::N_CORES]  # [m]
            padded = np.zeros(g["n_tiles"] * P, dtype=np.int16)
            padded[: g["m"]] = loc_c.astype(np.int16)
            blk = padded.reshape(g["n_tiles"], 8, 16)  # [tile, col, part]
            for j in range(g["n_tiles"]):
                cols[:, (t + j) * 8 : (t + j + 1) * 8] = blk[j].T
            t += g["n_tiles"]
        idx_arrs.append(np.ascontiguousarray(np.tile(cols, (8, 1))))  # [128, 8T]

    # projT host prep: transpose, fold EMB_SCALE, bf16, pad to d_pad,
    # chunk-major [128, kc*D_PROJ]
    pts = {}
    for b in range(N_BUCKETS):
        d, d_pad = d_emb[b], d_pads[b]
        ptb = np.zeros((d_pad, D_PROJ), dtype=np.float32)
        ptb[:d] = np.asarray(projs[b], dtype=np.float32).T * EMB_SCALE
        kc = d_pad // P
        ptb = ptb.reshape(kc, P, D_PROJ).transpose(1, 0, 2).reshape(P, kc * D_PROJ)
        pts[b] = np.ascontiguousarray(ptb.astype(ml_dtypes.bfloat16))

    nc = _build_graph(groups, T)

    in_maps = []
    for c in range(N_CORES):
        im = {"idx": idx_arrs[c]}
        for gi, g in enumerate(groups):
            im[f"emb{gi}"] = np.ascontiguousarray(
                embs_bf[g["bucket"]][g["base"] : g["base"] + g["rows"]]
            )
        for b in set(g["bucket"] for g in groups):
            im[f"pt{b}"] = pts[b]
        in_maps.append(im)

    res = run_bass_kernel_spmd(nc, in_maps, core_ids=list(range(N_CORES)))

    out_full = np.zeros((N, D_PROJ), dtype=np.float32)
    for c in range(N_CORES):
        shard = res.results[c]["out"]  # [R, D_PROJ]
        row = 0
        for g in groups:
            pos_c = g["pos"][c::N_CORES]  # [m]
            valid = pos_c >= 0
            out_full[pos_c[valid]] = shard[row : row + g["m"]][valid]
            row += g["m"]
    return out_full.reshape(*orig_shape, D_PROJ)


# revision 9
# speedup vs baseline: 1.1458x; 1.1458x over previous
"""Adaptive embedding lookup (4 vocab buckets, per-bucket projection) on 8 TRN2 cores.

Strategy: token-parallel SPMD. Tokens are bucketed by vocab range on host, each
bucket's vocab is split into <=32768-row chunks (int16 gather indices), each
group's token list is padded to a multiple of 8 and dealt round-robin so all 8
cores get identical static shapes. On device, each core:
  - dma_gather(transpose=True) pulls 128 tokens' bf16 embedding rows per tile,
    landing pre-transposed (embed dim on partitions, tokens on free dim) —
    directly usable as matmul lhsT, no on-chip transposes
  - matmuls against host-pre-transposed, EMB_SCALE-folded bf16 projections
  - writes its [rows, 1024] f32 output shard contiguously (group-major)
Host then inverse-permutes the 8 shards into the full [B, S, 1024] output.

Tables are converted to bf16 with embed dim zero-padded up to a multiple of
128 (dma_gather transpose needs 256B-aligned rows; zero dims contribute
nothing to the matmul).
"""
import sys

import numpy as np

if "/opt/trn_rl_repo" not in sys.path:
    sys.path.insert(0, "/opt/trn_rl_repo")

import ml_dtypes  # noqa: E402
from concourse import bacc, mybir, tile  # noqa: E402
from concourse.bass_utils import run_bass_kernel_spmd  # noqa: E402

N_CORES = 8
P = 128
CUTS = [0, 20000, 40000, 200000, 267735]
N_BUCKETS = 4
D_PROJ = 1024
EMB_SCALE = float(D_PROJ) ** 0.5
CHUNK = 32768  # max rows per gather table (int16 index range)

F32 = mybir.dt.float32
BF16 = mybir.dt.bfloat16
I16 = mybir.dt.int16


def _cdiv(a, b):
    return -(-a // b)


def _build_graph(groups, T):
    """groups: list of dicts with keys bucket, rows, d_pad, kc, m, n_tiles."""
    R = sum(g["m"] for g in groups)
    nc = bacc.Bacc(
        None,
        target_bir_lowering=False,
        debug=False,
        dynamic_dma_scratch_size=65536,
    )

    idx_p = nc.declare_dram_parameter("idx", [P, 8 * T], I16, isOutput=False)
    for gi, g in enumerate(groups):
        g["emb_p"] = nc.declare_dram_parameter(
            f"emb{gi}", [g["rows"], g["d_pad"]], BF16, isOutput=False
        )
    # one projT param per bucket, laid out [128, kc*D_PROJ]
    pt_p = {}
    kc_b = {}
    for g in groups:
        kc_b[g["bucket"]] = g["kc"]
    for b, kc in kc_b.items():
        pt_p[b] = nc.declare_dram_parameter(
            f"pt{b}", [P, kc * D_PROJ], BF16, isOutput=False
        )
    out_p = nc.declare_dram_parameter("out", [R, D_PROJ], F32, isOutput=True)

    with tile.TileContext(nc) as tc:
        with (
            tc.tile_pool(name="persist", bufs=1) as pp,
            tc.tile_pool(name="outs", bufs=3) as op,
            tc.tile_pool(name="ps_mm", bufs=6, space="PSUM") as ps_mm,
        ):
            idx_sb = pp.tile([P, 8 * T], I16)
            nc.sync.dma_start(out=idx_sb[:], in_=idx_p[:])

            pt_sb = {}
            for b, kc in kc_b.items():
                t_ = pp.tile([P, kc * D_PROJ], BF16, tag=f"pt{b}")
                nc.scalar.dma_start(out=t_[:], in_=pt_p[b][:])
                pt_sb[b] = t_

            # phase A: one gather per group into persistent SBUF staging
            t = 0
            for gi, g in enumerate(groups):
                ni = g["n_tiles"] * P
                gt = pp.tile([P, g["kc"], ni], BF16, tag=f"gt{gi}")
                nc.gpsimd.dma_gather(
                    out_ap=gt[:],
                    in_ap=g["emb_p"][:],
                    idxs_ap=idx_sb[:, t * 8 : (t + g["n_tiles"]) * 8],
                    num_idxs=ni,
                    num_idxs_reg=ni,
                    elem_size=g["d_pad"],
                    transpose=True,
                )
                g["gt"] = gt
                t += g["n_tiles"]

            # phase B: matmuls + copies + output DMA per 128-token tile
            row_start = 0
            for g in groups:
                b, kc, gt = g["bucket"], g["kc"], g["gt"]
                for j in range(g["n_tiles"]):
                    rows = min(P, g["m"] - j * P)
                    out_sb = op.tile([P, D_PROJ], F32, tag="o")
                    for h in range(D_PROJ // 512):
                        mm = ps_mm.tile([P, 512], F32, tag="mm")
                        for k in range(kc):
                            nc.tensor.matmul(
                                mm[:, :],
                                gt[:, k, j * P : (j + 1) * P],
                                pt_sb[b][:, k * D_PROJ + h * 512 : k * D_PROJ + (h + 1) * 512],
                                start=(k == 0),
                                stop=(k == kc - 1),
                            )
                        if h == 0:
                            nc.vector.tensor_copy(
                                out=out_sb[:, h * 512 : (h + 1) * 512], in_=mm[:, :]
                            )
                        else:
                            nc.scalar.activation(
                                out=out_sb[:, h * 512 : (h + 1) * 512],
                                in_=mm[:, :],
                                func=mybir.ActivationFunctionType.Copy,
                            )
                    nc.sync.dma_start(
                        out=out_p[row_start : row_start + rows, :],
                        in_=out_sb[:rows, :],
                    )
                    row_start += rows

    nc.compile()
    return nc


def kernel(inp, emb0, emb1, emb2, emb3, proj0, proj1, proj2, proj3):
    embs = [np.asarray(e, dtype=np.float32) for e in (emb0, emb1, emb2, emb3)]
    projs = [proj0, proj1, proj2, proj3]
    d_emb = [e.shape[1] for e in embs]
    v_emb = [e.shape[0] for e in embs]

    inp = np.asarray(inp)
    orig_shape = inp.shape
    flat = inp.reshape(-1).astype(np.int64)
    N = flat.shape[0]

    bucket = np.digitize(flat, CUTS[1:-1])  # 0..3
    local = flat - np.asarray(CUTS, dtype=np.int64)[bucket]

    # bf16 tables, embed dim zero-padded to a multiple of 128
    embs_bf = []
    d_pads = []
    for b in range(N_BUCKETS):
        d = d_emb[b]
        d_pad = max(P, _cdiv(d, P) * P)
        e = embs[b].astype(ml_dtypes.bfloat16)
        if d_pad != d:
            e = np.concatenate(
                [e, np.zeros((v_emb[b], d_pad - d), dtype=ml_dtypes.bfloat16)], axis=1
            )
        embs_bf.append(np.ascontiguousarray(e))
        d_pads.append(d_pad)

    # groups: (bucket, vocab chunk) — process small-projection buckets first
    # so the big proj0 load overlaps earlier compute
    groups = []
    for b in (1, 2, 3, 0):
        for base in range(0, v_emb[b], CHUNK):
            rows = min(CHUNK, v_emb[b] - base)
            sel = np.nonzero((bucket == b) & (local >= base) & (local < base + rows))[0]
            n = len(sel)
            npad = max(N_CORES, _cdiv(n, N_CORES) * N_CORES)
            pos_full = np.full(npad, -1, dtype=np.int64)
            loc_full = np.zeros(npad, dtype=np.int64)
            pos_full[:n] = sel
            loc_full[:n] = local[sel] - base
            m = npad // N_CORES
            groups.append(
                dict(
                    bucket=b,
                    base=base,
                    rows=rows,
                    d_pad=d_pads[b],
                    kc=d_pads[b] // P,
                    m=m,
                    n_tiles=_cdiv(m, P),
                    pos=pos_full,
                    loc=loc_full,
                )
            )
    T = sum(g["n_tiles"] for g in groups)

    # per-core int16 index arrays in dma_gather's wrapped layout:
    # gather slot i of tile t -> partition i%16, column t*8 + i//16,
    # 16-row block replicated 8x down the 128 partitions
    idx_arrs = []
    for c in range(N_CORES):
        cols = np.zeros((16, 8 * T), dtype=np.int16)
        t = 0
        for g in groups:
            loc_c = g["loc"][c::N_CORES]  # [m]
            padded = np.zeros(g["n_tiles"] * P, dtype=np.int16)
            padded[: g["m"]] = loc_c.astype(np.int16)
            blk = padded.reshape(g["n_tiles"], 8, 16)  # [tile, col, part]
            for j in range(g["n_tiles"]):
                cols[:, (t + j) * 8 : (t + j + 1) * 8] = blk[j].T
            t += g["n_tiles"]
        idx_arrs.append(np.ascontiguousarray(np.tile(cols, (8, 1))))  # [128, 8T]

    # projT host prep: transpose, fold EMB_SCALE, bf16, pad to d_pad,
    # chunk-major [128, kc*D_PROJ]
    pts = {}
    for b in range(N_BUCKETS):
        d, d_pad = d_emb[b], d_pads[b]
        ptb = np.zeros((d_pad, D_PROJ), dtype=np.float32)
        ptb[:d] = np.asarray(projs[b], dtype=np.float32).T * EMB_SCALE
        kc = d_pad // P
        ptb = ptb.reshape(kc, P, D_PROJ).transpose(1, 0, 2).reshape(P, kc * D_PROJ)
        pts[b] = np.ascontiguousarray(ptb.astype(ml_dtypes.bfloat16))

    nc = _build_graph(groups, T)

    in_maps = []
    for c in range(N_CORES):
        im = {"idx": idx_arrs[c]}
        for gi, g in enumerate(groups):
            im[f"emb{gi}"] = np.ascontiguousarray(
                embs_bf[g["bucket"]][g["base"] : g["base"] + g["rows"]]
            )
        for b in set(g["bucket"] for g in groups):
            im[f"pt{b}"] = pts[b]
        in_maps.append(im)

    res = run_bass_kernel_spmd(nc, in_maps, core_ids=list(range(N_CORES)))

    out_full = np.zeros((N, D_PROJ), dtype=np.float32)
    for c in range(N_CORES):
        shard = res.results[c]["out"]  # [R, D_PROJ]
        row = 0
        for g in groups:
            pos_c = g["pos"][c::N_CORES]  # [m]
            valid = pos_c >= 0
            out_full[pos_c[valid]] = shard[row : row + g["m"]][valid]
            row += g["m"]
    return out_full.reshape(*orig_shape, D_PROJ)


# revision 18
# speedup vs baseline: 1.4810x; 1.2926x over previous
"""Adaptive embedding lookup (4 vocab buckets, per-bucket projection) on 8 TRN2 cores.

Strategy: token-parallel SPMD. Tokens are bucketed by vocab range on host; each
bucket's token list is padded to a multiple of 8 and dealt round-robin so all 8
cores get identical static shapes. On device, each core:
  - indirect-DMA-gathers its tokens' embedding rows (f32 tables in DRAM,
    cast to bf16 in the DMA) — 128 tokens per gather, tokens on partitions
  - transposes each gathered [128, d] tile on the TensorEngine (matmul
    contracts over partitions, so embed-dim must be on partitions)
  - matmuls against host-pre-transposed, EMB_SCALE-folded bf16 projections
  - writes its [rows, 1024] f32 output shard contiguously (bucket-major),
    alternating the two HWDGE queues
Host then inverse-permutes the 8 shards into the full [B, S, 1024] output.
A burst of dummy matmuls at graph start warms the PE clock (HAM) while the
first gathers are in flight.
"""
import sys

import numpy as np

if "/opt/trn_rl_repo" not in sys.path:
    sys.path.insert(0, "/opt/trn_rl_repo")

import ml_dtypes  # noqa: E402
from concourse import bacc, bass, mybir, tile  # noqa: E402
from concourse.bass_utils import run_bass_kernel_spmd  # noqa: E402
from concourse.masks import make_identity  # noqa: E402

N_CORES = 8
P = 128
CUTS = [0, 20000, 40000, 200000, 267735]
N_BUCKETS = 4
D_PROJ = 1024
EMB_SCALE = float(D_PROJ) ** 0.5
BUCKET_ORDER = [0, 1, 2, 3]  # most PE work per gather first

F32 = mybir.dt.float32
BF16 = mybir.dt.bfloat16
I32 = mybir.dt.int32


def _cdiv(a, b):
    return -(-a // b)


def _build_graph(m, d_emb, v_emb, T):
    R = sum(m)
    nc = bacc.Bacc(None, target_bir_lowering=False, debug=False)

    idx_p = nc.declare_dram_parameter("idx", [P, T], I32, isOutput=False)
    emb_p = [
        nc.declare_dram_parameter(f"emb{b}", [v_emb[b], d_emb[b]], F32, isOutput=False)
        for b in range(N_BUCKETS)
    ]
    # projT params, one [128, 1024] bf16 chunk tile per 128 rows of embed dim
    pt_p = [
        nc.declare_dram_parameter(
            f"pt{b}", [_cdiv(d_emb[b], P), min(P, d_emb[b]), D_PROJ], BF16,
            isOutput=False,
        )
        for b in range(N_BUCKETS)
    ]
    out_p = nc.declare_dram_parameter("out", [R, D_PROJ], F32, isOutput=True)

    with tile.TileContext(nc) as tc:
        with (
            tc.tile_pool(name="persist", bufs=1) as pp,
            tc.tile_pool(name="gather", bufs=4) as gp,
            tc.tile_pool(name="lhsT", bufs=3) as lp,
            tc.tile_pool(name="outs", bufs=4) as op,
            tc.tile_pool(name="ps_tr", bufs=3, space="PSUM") as ps_tr,
            tc.tile_pool(name="ps_mm", bufs=4, space="PSUM") as ps_mm,
            tc.tile_pool(name="ps_warm", bufs=1, space="PSUM") as ps_warm,
        ):
            ident = pp.tile([P, P], F32)
            make_identity(nc, ident[:])

            # PE warmup: dummy matmuls with no data deps keep the HAM busy
            # window alive while the first gathers land
            warm = pp.tile([P, 512], BF16, tag="warm")
            nc.vector.memset(warm[:], 0)
            wps = ps_warm.tile([P, 512], F32, tag="warm_ps")
            for _ in range(22):
                nc.tensor.matmul(wps[:], warm[:, :P], warm[:], start=True, stop=True)

            idx_sb = pp.tile([P, T], I32)
            nc.sync.dma_start(out=idx_sb[:], in_=idx_p[:])

            pt_sb = []
            for b in range(N_BUCKETS):
                kc = _cdiv(d_emb[b], P)
                rows = min(P, d_emb[b])
                chunks = []
                for k in range(kc):
                    t_ = pp.tile([P, D_PROJ], BF16, tag=f"pt{b}_{k}")
                    nc.scalar.dma_start(out=t_[:rows, :], in_=pt_p[b][k, :, :])
                    chunks.append(t_)
                pt_sb.append(chunks)

            t_of = {}
            t = 0
            for b in BUCKET_ORDER:
                t_of[b] = t
                t += _cdiv(m[b], P)

            row_of = {}
            row = 0
            for b in BUCKET_ORDER:
                row_of[b] = row
                row += m[b]

            # interleave heavy (b0/b1) tiles among light (b2/b3) ones so the
            # TensorEngine never idles long enough for HAM to re-throttle
            tiles = []
            for b in BUCKET_ORDER:
                for j in range(_cdiv(m[b], P)):
                    tiles.append((b, j))
            heavy = [x for x in tiles if x[0] in (0, 1)]
            light = [x for x in tiles if x[0] not in (0, 1)]
            order = []
            li = 0
            for hi, h_ in enumerate(heavy):
                order.append(h_)
                take = min(3, len(light) - li)
                order.extend(light[li : li + take])
                li += take
            order.extend(light[li:])

            dma_engines = [nc.sync, nc.scalar]
            n_dma = 0
            for b, j in order:
                d = d_emb[b]
                kc = _cdiv(d, P)
                t = t_of[b] + j
                rows = min(P, m[b] - j * P)
                g = gp.tile([P, d], F32, tag=f"g{b}")
                nc.gpsimd.indirect_dma_start(
                    out=g[:],
                    out_offset=None,
                    in_=emb_p[b][:],
                    in_offset=bass.IndirectOffsetOnAxis(
                        ap=idx_sb[:, t : t + 1], axis=0
                    ),
                )
                lhsT = lp.tile([P, kc * P], BF16, tag=f"l{b}")
                for k in range(kc):
                    cw = min(P, d - k * P)
                    trp = ps_tr.tile([P, P], F32, tag="tr")
                    nc.tensor.transpose(
                        out=trp[:cw, :P],
                        in_=g[:, k * P : k * P + cw],
                        identity=ident[:],
                    )
                    nc.vector.tensor_copy(
                        out=lhsT[:cw, k * P : (k + 1) * P], in_=trp[:cw, :P]
                    )
                out_sb = op.tile([P, D_PROJ], F32, tag="o")
                for h in range(D_PROJ // 512):
                    mm = ps_mm.tile([P, 512], F32, tag="mm")
                    for k in range(kc):
                        cw = min(P, d - k * P)
                        nc.tensor.matmul(
                            mm[:, :],
                            lhsT[:cw, k * P : (k + 1) * P],
                            pt_sb[b][k][:cw, h * 512 : (h + 1) * 512],
                            start=(k == 0),
                            stop=(k == kc - 1),
                        )
                    if h == 0:
                        nc.vector.tensor_copy(
                            out=out_sb[:, h * 512 : (h + 1) * 512], in_=mm[:, :]
                        )
                    else:
                        nc.scalar.activation(
                            out=out_sb[:, h * 512 : (h + 1) * 512],
                            in_=mm[:, :],
                            func=mybir.ActivationFunctionType.Copy,
                        )
                r0 = row_of[b] + j * P
                dma_engines[n_dma % 2].dma_start(
                    out=out_p[r0 : r0 + rows, :],
                    in_=out_sb[:rows, :],
                )
                n_dma += 1

    nc.compile()
    return nc


def kernel(inp, emb0, emb1, emb2, emb3, proj0, proj1, proj2, proj3):
    embs = [np.ascontiguousarray(e, dtype=np.float32) for e in (emb0, emb1, emb2, emb3)]
    projs = [proj0, proj1, proj2, proj3]
    d_emb = [e.shape[1] for e in embs]
    v_emb = [e.shape[0] for e in embs]

    inp = np.asarray(inp)
    orig_shape = inp.shape
    flat = inp.reshape(-1).astype(np.int64)
    N = flat.shape[0]

    bucket = np.digitize(flat, CUTS[1:-1])  # 0..3
    local = flat - np.asarray(CUTS, dtype=np.int64)[bucket]

    pos_pad, loc_pad, m = [], [], []
    for b in range(N_BUCKETS):
        pos = np.nonzero(bucket == b)[0]
        loc = np.clip(local[pos], 0, v_emb[b] - 1)
        n = len(pos)
        npad = max(N_CORES, _cdiv(n, N_CORES) * N_CORES)
        pos_full = np.full(npad, -1, dtype=np.int64)
        loc_full = np.zeros(npad, dtype=np.int64)
        pos_full[:n] = pos
        loc_full[:n] = loc
        pos_pad.append(pos_full)
        loc_pad.append(loc_full)
        m.append(npad // N_CORES)

    n_tiles = [_cdiv(mb, P) for mb in m]
    T = sum(n_tiles[b] for b in BUCKET_ORDER)

    # per-core index arrays [P, T]: column t = 128 table-row indices for
    # gather tile t (bucket-major in BUCKET_ORDER, zero-padded)
    idx_arrs = []
    for c in range(N_CORES):
        cols = np.zeros((T, P), dtype=np.int32)
        t = 0
        for b in BUCKET_ORDER:
            locs_c = loc_pad[b][c::N_CORES]
            padded = np.zeros(n_tiles[b] * P, dtype=np.int32)
            padded[: m[b]] = locs_c.astype(np.int32)
            cols[t : t + n_tiles[b]] = padded.reshape(n_tiles[b], P)
            t += n_tiles[b]
        idx_arrs.append(np.ascontiguousarray(cols.T))

    # projT host prep: transpose, fold EMB_SCALE, bf16, [kc, <=128, 1024]
    pts = []
    for b in range(N_BUCKETS):
        d = d_emb[b]
        kc = _cdiv(d, P)
        ptb = np.asarray(projs[b], dtype=np.float32).T * EMB_SCALE  # [d, D_PROJ]
        ptb = ptb.reshape(kc, min(P, d), D_PROJ)
        pts.append(np.ascontiguousarray(ptb.astype(ml_dtypes.bfloat16)))

    nc = _build_graph(m, d_emb, v_emb, T)

    in_maps = []
    for c in range(N_CORES):
        im = {"idx": idx_arrs[c]}
        for b in range(N_BUCKETS):
            im[f"emb{b}"] = embs[b]
            im[f"pt{b}"] = pts[b]
        in_maps.append(im)

    res = run_bass_kernel_spmd(nc, in_maps, core_ids=list(range(N_CORES)))

    out_full = np.zeros((N, D_PROJ), dtype=np.float32)
    for c in range(N_CORES):
        shard = res.results[c]["out"]
        row = 0
        for b in BUCKET_ORDER:
            pos_c = pos_pad[b][c::N_CORES]
            valid = pos_c >= 0
            out_full[pos_c[valid]] = shard[row : row + m[b]][valid]
            row += m[b]
    return out_full.reshape(*orig_shape, D_PROJ)


# revision 20
# speedup vs baseline: 1.5740x; 1.0627x over previous
"""Adaptive embedding lookup (4 vocab buckets, per-bucket projection) on 8 TRN2 cores.

Strategy: token-parallel SPMD. Tokens are bucketed by vocab range on host; each
bucket's token list is padded to a multiple of 8 and dealt round-robin so all 8
cores get identical static shapes. On device, each core:
  - indirect-DMA-gathers its tokens' embedding rows (f32 tables in DRAM,
    cast to bf16 in the DMA) — 128 tokens per gather, tokens on partitions
  - transposes each gathered [128, d] tile on the TensorEngine (matmul
    contracts over partitions, so embed-dim must be on partitions)
  - matmuls against host-pre-transposed, EMB_SCALE-folded bf16 projections
  - writes its [rows, 1024] f32 output shard contiguously (bucket-major),
    alternating the two HWDGE queues
Host then inverse-permutes the 8 shards into the full [B, S, 1024] output.
A burst of dummy matmuls at graph start warms the PE clock (HAM) while the
first gathers are in flight.
"""
import sys

import numpy as np

if "/opt/trn_rl_repo" not in sys.path:
    sys.path.insert(0, "/opt/trn_rl_repo")

import ml_dtypes  # noqa: E402
from concourse import bacc, bass, mybir, tile  # noqa: E402
from concourse.bass_utils import run_bass_kernel_spmd  # noqa: E402
from concourse.masks import make_identity  # noqa: E402

N_CORES = 8
P = 128
CUTS = [0, 20000, 40000, 200000, 267735]
N_BUCKETS = 4
D_PROJ = 1024
EMB_SCALE = float(D_PROJ) ** 0.5
BUCKET_ORDER = [0, 1, 2, 3]  # most PE work per gather first

F32 = mybir.dt.float32
BF16 = mybir.dt.bfloat16
I32 = mybir.dt.int32


def _cdiv(a, b):
    return -(-a // b)


def _build_graph(m, d_emb, v_emb, T):
    R = sum(m)
    nc = bacc.Bacc(None, target_bir_lowering=False, debug=False)

    idx_p = nc.declare_dram_parameter("idx", [P, T], I32, isOutput=False)
    emb_p = [
        nc.declare_dram_parameter(f"emb{b}", [v_emb[b], d_emb[b]], F32, isOutput=False)
        for b in range(N_BUCKETS)
    ]
    # projT params, one [128, 1024] bf16 chunk tile per 128 rows of embed dim
    pt_p = [
        nc.declare_dram_parameter(
            f"pt{b}", [_cdiv(d_emb[b], P), min(P, d_emb[b]), D_PROJ], BF16,
            isOutput=False,
        )
        for b in range(N_BUCKETS)
    ]
    out_p = nc.declare_dram_parameter("out", [R, D_PROJ], F32, isOutput=True)

    with tile.TileContext(nc) as tc:
        with (
            tc.tile_pool(name="persist", bufs=1) as pp,
            tc.tile_pool(name="gather", bufs=6) as gp,
            tc.tile_pool(name="lhsT", bufs=3) as lp,
            tc.tile_pool(name="outs", bufs=6) as op,
            tc.tile_pool(name="ps_tr", bufs=3, space="PSUM") as ps_tr,
            tc.tile_pool(name="ps_mm", bufs=4, space="PSUM") as ps_mm,
            tc.tile_pool(name="ps_warm", bufs=1, space="PSUM") as ps_warm,
        ):
            ident = pp.tile([P, P], F32)
            make_identity(nc, ident[:])

            # PE warmup: dummy matmuls with no data deps keep the HAM busy
            # window alive while the first gathers land
            warm = pp.tile([P, 512], BF16, tag="warm")
            nc.vector.memset(warm[:], 0)
            wps = ps_warm.tile([P, 512], F32, tag="warm_ps")
            for _ in range(22):
                nc.tensor.matmul(wps[:], warm[:, :P], warm[:], start=True, stop=True)

            idx_sb = pp.tile([P, T], I32)
            nc.sync.dma_start(out=idx_sb[:], in_=idx_p[:])

            pt_sb = []
            for b in range(N_BUCKETS):
                kc = _cdiv(d_emb[b], P)
                rows = min(P, d_emb[b])
                chunks = []
                for k in range(kc):
                    t_ = pp.tile([P, D_PROJ], BF16, tag=f"pt{b}_{k}")
                    nc.scalar.dma_start(out=t_[:rows, :], in_=pt_p[b][k, :, :])
                    chunks.append(t_)
                pt_sb.append(chunks)

            t_of = {}
            t = 0
            for b in BUCKET_ORDER:
                t_of[b] = t
                t += _cdiv(m[b], P)

            row_of = {}
            row = 0
            for b in BUCKET_ORDER:
                row_of[b] = row
                row += m[b]

            # interleave heavy (b0/b1) tiles among light (b2/b3) ones so the
            # TensorEngine never idles long enough for HAM to re-throttle
            tiles = []
            for b in BUCKET_ORDER:
                for j in range(_cdiv(m[b], P)):
                    tiles.append((b, j))
            heavy = [x for x in tiles if x[0] in (0, 1)]
            light = [x for x in tiles if x[0] not in (0, 1)]
            order = []
            li = 0
            for hi, h_ in enumerate(heavy):
                order.append(h_)
                take = min(3, len(light) - li)
                order.extend(light[li : li + take])
                li += take
            order.extend(light[li:])

            dma_engines = [nc.sync, nc.scalar]
            n_dma = 0
            for b, j in order:
                d = d_emb[b]
                kc = _cdiv(d, P)
                t = t_of[b] + j
                rows = min(P, m[b] - j * P)
                g = gp.tile([P, d], F32, tag=f"g{b}")
                nc.gpsimd.indirect_dma_start(
                    out=g[:],
                    out_offset=None,
                    in_=emb_p[b][:],
                    in_offset=bass.IndirectOffsetOnAxis(
                        ap=idx_sb[:, t : t + 1], axis=0
                    ),
                )
                lhsT = lp.tile([P, kc * P], BF16, tag=f"l{b}")
                for k in range(kc):
                    cw = min(P, d - k * P)
                    trp = ps_tr.tile([P, P], F32, tag="tr")
                    nc.tensor.transpose(
                        out=trp[:cw, :P],
                        in_=g[:, k * P : k * P + cw],
                        identity=ident[:],
                    )
                    nc.vector.tensor_copy(
                        out=lhsT[:cw, k * P : (k + 1) * P], in_=trp[:cw, :P]
                    )
                out_sb = op.tile([P, D_PROJ], F32, tag="o")
                for h in range(D_PROJ // 512):
                    mm = ps_mm.tile([P, 512], F32, tag="mm")
                    for k in range(kc):
                        cw = min(P, d - k * P)
                        nc.tensor.matmul(
                            mm[:, :],
                            lhsT[:cw, k * P : (k + 1) * P],
                            pt_sb[b][k][:cw, h * 512 : (h + 1) * 512],
                            start=(k == 0),
                            stop=(k == kc - 1),
                        )
                    if h == 0 and b in (0, 1):
                        # DVE takes the heavy tiles' first half; ACT the rest,
                        # keeping DVE free to feed the PE with lhsT casts
                        nc.vector.tensor_copy(
                            out=out_sb[:, h * 512 : (h + 1) * 512], in_=mm[:, :]
                        )
                    else:
                        nc.scalar.activation(
                            out=out_sb[:, h * 512 : (h + 1) * 512],
                            in_=mm[:, :],
                            func=mybir.ActivationFunctionType.Copy,
                        )
                r0 = row_of[b] + j * P
                nc.sync.dma_start(
                    out=out_p[r0 : r0 + rows, :],
                    in_=out_sb[:rows, :],
                )
                n_dma += 1

    nc.compile()
    return nc


def kernel(inp, emb0, emb1, emb2, emb3, proj0, proj1, proj2, proj3):
    embs = [np.ascontiguousarray(e, dtype=np.float32) for e in (emb0, emb1, emb2, emb3)]
    projs = [proj0, proj1, proj2, proj3]
    d_emb = [e.shape[1] for e in embs]
    v_emb = [e.shape[0] for e in embs]

    inp = np.asarray(inp)
    orig_shape = inp.shape
    flat = inp.reshape(-1).astype(np.int64)
    N = flat.shape[0]

    bucket = np.digitize(flat, CUTS[1:-1])  # 0..3
    local = flat - np.asarray(CUTS, dtype=np.int64)[bucket]

    pos_pad, loc_pad, m = [], [], []
    for b in range(N_BUCKETS):
        pos = np.nonzero(bucket == b)[0]
        loc = np.clip(local[pos], 0, v_emb[b] - 1)
        n = len(pos)
        npad = max(N_CORES, _cdiv(n, N_CORES) * N_CORES)
        pos_full = np.full(npad, -1, dtype=np.int64)
        loc_full = np.zeros(npad, dtype=np.int64)
        pos_full[:n] = pos
        loc_full[:n] = loc
        pos_pad.append(pos_full)
        loc_pad.append(loc_full)
        m.append(npad // N_CORES)

    n_tiles = [_cdiv(mb, P) for mb in m]
    T = sum(n_tiles[b] for b in BUCKET_ORDER)

    # per-core index arrays [P, T]: column t = 128 table-row indices for
    # gather tile t (bucket-major in BUCKET_ORDER, zero-padded)
    idx_arrs = []
    for c in range(N_CORES):
        cols = np.zeros((T, P), dtype=np.int32)
        t = 0
        for b in BUCKET_ORDER:
            locs_c = loc_pad[b][c::N_CORES]
            padded = np.zeros(n_tiles[b] * P, dtype=np.int32)
            padded[: m[b]] = locs_c.astype(np.int32)
            cols[t : t + n_tiles[b]] = padded.reshape(n_tiles[b], P)
            t += n_tiles[b]
        idx_arrs.append(np.ascontiguousarray(cols.T))

    # projT host prep: transpose, fold EMB_SCALE, bf16, [kc, <=128, 1024]
    pts = []
    for b in range(N_BUCKETS):
        d = d_emb[b]
        kc = _cdiv(d, P)
        ptb = np.asarray(projs[b], dtype=np.float32).T * EMB_SCALE  # [d, D_PROJ]
        ptb = ptb.reshape(kc, min(P, d), D_PROJ)
        pts.append(np.ascontiguousarray(ptb.astype(ml_dtypes.bfloat16)))

    nc = _build_graph(m, d_emb, v_emb, T)

    in_maps = []
    for c in range(N_CORES):
        im = {"idx": idx_arrs[c]}
        for b in range(N_BUCKETS):
            im[f"emb{b}"] = embs[b]
            im[f"pt{b}"] = pts[b]
        in_maps.append(im)

    res = run_bass_kernel_spmd(nc, in_maps, core_ids=list(range(N_CORES)))

    out_full = np.zeros((N, D_PROJ), dtype=np.float32)
    for c in range(N_CORES):
        shard = res.results[c]["out"]
        row = 0
        for b in BUCKET_ORDER:
            pos_c = pos_pad[b][c::N_CORES]
            valid = pos_c >= 0
            out_full[pos_c[valid]] = shard[row : row + m[b]][valid]
            row += m[b]
    return out_full.reshape(*orig_shape, D_PROJ)


# revision 25
# speedup vs baseline: 1.6915x; 1.0747x over previous
"""Adaptive embedding lookup (4 vocab buckets, per-bucket projection) on 8 TRN2 cores.

Strategy: token-parallel SPMD. Tokens are bucketed by vocab range on host; each
bucket's token list is padded to a multiple of 8 and dealt round-robin so all 8
cores get identical static shapes. On device, each core:
  - indirect-DMA-gathers its tokens' embedding rows (f32 tables in DRAM,
    cast to bf16 in the DMA) — 128 tokens per gather, tokens on partitions
  - transposes each gathered [128, d] tile on the TensorEngine (matmul
    contracts over partitions, so embed-dim must be on partitions)
  - matmuls against host-pre-transposed, EMB_SCALE-folded bf16 projections
  - writes its [rows, 1024] f32 output shard contiguously (bucket-major),
    alternating the two HWDGE queues
Host then inverse-permutes the 8 shards into the full [B, S, 1024] output.
A burst of dummy matmuls at graph start warms the PE clock (HAM) while the
first gathers are in flight.
"""
import sys

import numpy as np

if "/opt/trn_rl_repo" not in sys.path:
    sys.path.insert(0, "/opt/trn_rl_repo")

import ml_dtypes  # noqa: E402
from concourse import bacc, bass, mybir, tile  # noqa: E402
from concourse.bass_utils import run_bass_kernel_spmd  # noqa: E402
from concourse.masks import make_identity  # noqa: E402

N_CORES = 8
P = 128
CUTS = [0, 20000, 40000, 200000, 267735]
N_BUCKETS = 4
D_PROJ = 1024
EMB_SCALE = float(D_PROJ) ** 0.5
BUCKET_ORDER = [0, 1, 2, 3]  # most PE work per gather first

F32 = mybir.dt.float32
BF16 = mybir.dt.bfloat16
I32 = mybir.dt.int32


def _cdiv(a, b):
    return -(-a // b)


def _build_graph(m, d_emb, v_emb, T):
    R = sum(m)
    nc = bacc.Bacc(None, target_bir_lowering=False, debug=False)

    idx_p = nc.declare_dram_parameter("idx", [P, T], I32, isOutput=False)
    emb_p = [
        nc.declare_dram_parameter(f"emb{b}", [v_emb[b], d_emb[b]], F32, isOutput=False)
        for b in range(N_BUCKETS)
    ]
    # projT params, one [128, 1024] bf16 chunk tile per 128 rows of embed dim
    pt_p = [
        nc.declare_dram_parameter(
            f"pt{b}", [_cdiv(d_emb[b], P), min(P, d_emb[b]), D_PROJ], BF16,
            isOutput=False,
        )
        for b in range(N_BUCKETS)
    ]
    out_p = nc.declare_dram_parameter("out", [R, D_PROJ], F32, isOutput=True)

    with tile.TileContext(nc) as tc:
        with (
            tc.tile_pool(name="persist", bufs=1) as pp,
            tc.tile_pool(name="gather", bufs=6) as gp,
            tc.tile_pool(name="lhsT", bufs=3) as lp,
            tc.tile_pool(name="outs", bufs=6) as op,
            tc.tile_pool(name="ps_tr", bufs=3, space="PSUM") as ps_tr,
            tc.tile_pool(name="ps_mm", bufs=2, space="PSUM") as ps_mm,
            tc.tile_pool(name="ps_warm", bufs=1, space="PSUM") as ps_warm,
        ):
            ident = pp.tile([P, P], F32)
            make_identity(nc, ident[:])

            # PE warmup: dummy matmuls with no data deps keep the HAM busy
            # window alive while the first gathers land
            warm = pp.tile([P, 512], BF16, tag="warm")
            nc.vector.memset(warm[:], 0)
            wps = ps_warm.tile([P, 512], F32, tag="warm_ps")
            for _ in range(10):
                nc.tensor.matmul(wps[:], warm[:, :P], warm[:], start=True, stop=True)

            idx_sb = pp.tile([P, T], I32)
            nc.sync.dma_start(out=idx_sb[:], in_=idx_p[:])

            pt_sb = []
            for b in range(N_BUCKETS):
                kc = _cdiv(d_emb[b], P)
                rows = min(P, d_emb[b])
                chunks = []
                for k in range(kc):
                    t_ = pp.tile([P, D_PROJ], BF16, tag=f"pt{b}_{k}")
                    nc.scalar.dma_start(out=t_[:rows, :], in_=pt_p[b][k, :, :])
                    chunks.append(t_)
                pt_sb.append(chunks)

            t_of = {}
            t = 0
            for b in BUCKET_ORDER:
                t_of[b] = t
                t += _cdiv(m[b], P)

            row_of = {}
            row = 0
            for b in BUCKET_ORDER:
                row_of[b] = row
                row += m[b]

            # interleave heavy (b0/b1) tiles among light (b2/b3) ones so the
            # TensorEngine never idles long enough for HAM to re-throttle
            tiles = []
            for b in BUCKET_ORDER:
                for j in range(_cdiv(m[b], P)):
                    tiles.append((b, j))
            heavy = [x for x in tiles if x[0] in (0, 1)]
            light = [x for x in tiles if x[0] not in (0, 1)]
            order = []
            li = 0
            for hi, h_ in enumerate(heavy):
                order.append(h_)
                take = min(3, len(light) - li)
                order.extend(light[li : li + take])
                li += take
            order.extend(light[li:])

            dma_engines = [nc.sync, nc.scalar]
            n_dma = 0
            for b, j in order:
                d = d_emb[b]
                kc = _cdiv(d, P)
                t = t_of[b] + j
                rows = min(P, m[b] - j * P)
                g = gp.tile([P, d], F32, tag=f"g{b}")
                nc.gpsimd.indirect_dma_start(
                    out=g[:],
                    out_offset=None,
                    in_=emb_p[b][:],
                    in_offset=bass.IndirectOffsetOnAxis(
                        ap=idx_sb[:, t : t + 1], axis=0
                    ),
                )
                lhsT = lp.tile([P, kc * P], BF16, tag=f"l{b}")
                for k in range(kc):
                    cw = min(P, d - k * P)
                    trp = ps_tr.tile([P, P], F32, tag="tr")
                    nc.tensor.transpose(
                        out=trp[:cw, :P],
                        in_=g[:, k * P : k * P + cw],
                        identity=ident[:],
                    )
                    nc.vector.tensor_copy(
                        out=lhsT[:cw, k * P : (k + 1) * P], in_=trp[:cw, :P]
                    )
                out_sb = op.tile([P, D_PROJ], F32, tag="o")
                # k-outer / h-inner: consecutive matmuls hit alternating PSUM
                # banks, so each one's fill overlaps the previous one's drain
                mm0 = ps_mm.tile([P, 512], F32, tag="mm0")
                mm1 = ps_mm.tile([P, 512], F32, tag="mm1")
                mms = [mm0, mm1]
                for k in range(kc):
                    cw = min(P, d - k * P)
                    for h in range(2):
                        nc.tensor.matmul(
                            mms[h][:, :],
                            lhsT[:cw, k * P : (k + 1) * P],
                            pt_sb[b][k][:cw, h * 512 : (h + 1) * 512],
                            start=(k == 0),
                            stop=(k == kc - 1),
                        )
                for h in range(2):
                    # alternate which engine takes which half per tile
                    eng_is_vec = (n_dma + h) % 2 == 0
                    if eng_is_vec:
                        nc.vector.tensor_copy(
                            out=out_sb[:, h * 512 : (h + 1) * 512], in_=mms[h][:, :]
                        )
                    else:
                        nc.scalar.activation(
                            out=out_sb[:, h * 512 : (h + 1) * 512],
                            in_=mms[h][:, :],
                            func=mybir.ActivationFunctionType.Copy,
                        )
                r0 = row_of[b] + j * P
                nc.sync.dma_start(
                    out=out_p[r0 : r0 + rows, :],
                    in_=out_sb[:rows, :],
                )
                n_dma += 1

    nc.compile()
    return nc


def kernel(inp, emb0, emb1, emb2, emb3, proj0, proj1, proj2, proj3):
    embs = [np.ascontiguousarray(e, dtype=np.float32) for e in (emb0, emb1, emb2, emb3)]
    projs = [proj0, proj1, proj2, proj3]
    d_emb = [e.shape[1] for e in embs]
    v_emb = [e.shape[0] for e in embs]

    inp = np.asarray(inp)
    orig_shape = inp.shape
    flat = inp.reshape(-1).astype(np.int64)
    N = flat.shape[0]

    bucket = np.digitize(flat, CUTS[1:-1])  # 0..3
    local = flat - np.asarray(CUTS, dtype=np.int64)[bucket]

    pos_pad, loc_pad, m = [], [], []
    for b in range(N_BUCKETS):
        pos = np.nonzero(bucket == b)[0]
        loc = np.clip(local[pos], 0, v_emb[b] - 1)
        srt = np.argsort(loc, kind="stable")  # row-sorted gathers: HBM locality
        pos, loc = pos[srt], loc[srt]
        n = len(pos)
        npad = max(N_CORES, _cdiv(n, N_CORES) * N_CORES)
        pos_full = np.full(npad, -1, dtype=np.int64)
        loc_full = np.zeros(npad, dtype=np.int64)
        pos_full[:n] = pos
        loc_full[:n] = loc
        pos_pad.append(pos_full)
        loc_pad.append(loc_full)
        m.append(npad // N_CORES)

    n_tiles = [_cdiv(mb, P) for mb in m]
    T = sum(n_tiles[b] for b in BUCKET_ORDER)

    # per-core index arrays [P, T]: column t = 128 table-row indices for
    # gather tile t (bucket-major in BUCKET_ORDER, zero-padded)
    idx_arrs = []
    for c in range(N_CORES):
        cols = np.zeros((T, P), dtype=np.int32)
        t = 0
        for b in BUCKET_ORDER:
            locs_c = loc_pad[b][c::N_CORES]
            padded = np.zeros(n_tiles[b] * P, dtype=np.int32)
            padded[: m[b]] = locs_c.astype(np.int32)
            cols[t : t + n_tiles[b]] = padded.reshape(n_tiles[b], P)
            t += n_tiles[b]
        idx_arrs.append(np.ascontiguousarray(cols.T))

    # projT host prep: transpose, fold EMB_SCALE, bf16, [kc, <=128, 1024]
    pts = []
    for b in range(N_BUCKETS):
        d = d_emb[b]
        kc = _cdiv(d, P)
        ptb = np.asarray(projs[b], dtype=np.float32).T * EMB_SCALE  # [d, D_PROJ]
        ptb = ptb.reshape(kc, min(P, d), D_PROJ)
        pts.append(np.ascontiguousarray(ptb.astype(ml_dtypes.bfloat16)))

    nc = _build_graph(m, d_emb, v_emb, T)

    in_maps = []
    for c in range(N_CORES):
        im = {"idx": idx_arrs[c]}
        for b in range(N_BUCKETS):
            im[f"emb{b}"] = embs[b]
            im[f"pt{b}"] = pts[b]
        in_maps.append(im)

    res = run_bass_kernel_spmd(nc, in_maps, core_ids=list(range(N_CORES)))

    out_full = np.zeros((N, D_PROJ), dtype=np.float32)
    for c in range(N_CORES):
        shard = res.results[c]["out"]
        row = 0
        for b in BUCKET_ORDER:
            pos_c = pos_pad[b][c::N_CORES]
            valid = pos_c >= 0
            out_full[pos_c[valid]] = shard[row : row + m[b]][valid]
            row += m[b]
    return out_full.reshape(*orig_shape, D_PROJ)
